# revision 1
# baseline (speedup 1.0000x reference)
"""2-layer GCN encoder on 8 Trainium2 NeuronCores.

Strategy (dst-sharded graph parallel, 3 SPMD launches):
  L1: per core, g1 = dinv * (x_shard @ W1)             [dense, tiny]
  L2: per core, gather g1[src] rows (dma_gather, int16 src windows),
      segmented-sum via fp8 one-hot indicator matmuls into PSUM per
      128-dst tile, evict with *dinv[dst] (+b1, relu), PE-transpose,
      @W2, *dinv -> g2 shard                            [memory bound]
  L3: same aggregation on g2 -> final out shard
Host relays the full g tables between launches (the all-to-all) and
precomputes the edge structure (sort, windows, indicators) in numpy.
"""

import sys

sys.path.insert(0, "/opt/trn_rl_repo")

import ml_dtypes
import numpy as np

from concourse import bacc, bass, library_config, mybir, tile
from concourse.bass_utils import run_bass_kernel_spmd

N_NODES = 100000
IN_C = 128
H2 = 128  # 2*hid
HID = 64
NCORES = 8
SHARD = N_NODES // NCORES  # 12500
P = 128
NT = 98  # ceil(12500/128) dst tiles per core
SLOTS = NT * P  # 12544
TBATCH = 4  # dst tiles per psum batch
NB = 25  # ceil(NT/TBATCH) batches (98 -> 25, last has 2)
WIN = 25000  # src window size (int16 gather indices)
NW = 4
MAXG_CALL = 8  # groups per dma_gather call (1024 idx; larger crashes the DMA ring)

BF16 = ml_dtypes.bfloat16
FP8 = ml_dtypes.float8_e4m3


# ---------------------------------------------------------------- host prep
def _prepare(edge_index):
    src = np.asarray(edge_index[0], dtype=np.int64)
    dst = np.asarray(edge_index[1], dtype=np.int64)
    deg = np.bincount(dst, minlength=N_NODES).astype(np.float32) + 1.0
    dinv = 1.0 / np.sqrt(deg)

    # per-core edge lists incl self-loops
    loops = np.arange(N_NODES, dtype=np.int64)
    src_all = np.concatenate([src, loops])
    dst_all = np.concatenate([dst, loops])
    core_of = dst_all // SHARD

    cores = []
    for k in range(NCORES):
        m = core_of == k
        es, ed = src_all[m], dst_all[m] - k * SHARD
        tile_id = ed // P
        win_id = es // WIN
        order = np.lexsort((ed, win_id, tile_id))
        es, ed, tile_id, win_id = es[order], ed[order], tile_id[order], win_id[order]
        # counts per (tile, window)
        key = tile_id * NW + win_id
        cnt = np.bincount(key, minlength=NT * NW).reshape(NT, NW)
        cores.append(dict(es=es, ed=ed, cnt=cnt))

    # shared group structure: G[t, w] = max_k ceil(cnt/128)
    cnts = np.stack([c["cnt"] for c in cores])  # [8, NT, NW]
    G = (-(-cnts // P)).max(axis=0)  # [NT, NW] ceil then max over cores
    # Build the global group order: for batch B: for w: for t in B: groups
    group_tile = []  # tile of each group, in global order
    group_w = []
    calls = []  # (w, goff, gc) per dma_gather call; calls never cross (t, w)
    for b in range(NB):
        tiles_b = range(b * TBATCH, min((b + 1) * TBATCH, NT))
        for w in range(NW):
            seg = []
            for t in tiles_b:
                seg += [t] * int(G[t, w])
            goff = len(group_tile)
            for cstart in range(0, len(seg), MAXG_CALL):
                gc = min(MAXG_CALL, len(seg) - cstart)
                calls.append((w, goff + cstart, gc))
            group_tile += seg
            group_w += [w] * len(seg)
    group_tile = np.array(group_tile, dtype=np.int64)
    group_w = np.array(group_w, dtype=np.int64)
    GT = len(group_tile)

    # start/stop flags per group (first/last group of its tile)
    start_flag = np.zeros(GT, dtype=bool)
    stop_flag = np.zeros(GT, dtype=bool)
    seen = set()
    for g in range(GT):
        t = group_tile[g]
        if t not in seen:
            start_flag[g] = True
            seen.add(t)
    seen = set()
    for g in range(GT - 1, -1, -1):
        t = group_tile[g]
        if t not in seen:
            stop_flag[g] = True
            seen.add(t)

    # group start offset of each (t, w) run in the global group order
    g_of_run = {}
    gi = 0
    for b in range(NB):
        tiles_b = range(b * TBATCH, min((b + 1) * TBATCH, NT))
        for w in range(NW):
            for t in tiles_b:
                g_of_run[(t, w)] = gi
                gi += G[t, w]
    assert gi == GT

    # per-core idx16 + indicator arrays
    # slot layout: group g occupies idx positions [g*128, (g+1)*128)
    per_core = []
    for k in range(NCORES):
        c = cores[k]
        es, ed, cnt = c["es"], c["ed"], c["cnt"]
        # offsets of each (t, w) run inside the sorted edge list
        run_len = cnt.reshape(-1)
        run_off = np.concatenate([[0], np.cumsum(run_len)])[:-1].reshape(NT, NW)

        # pads are -1: always trailing within their (t, w) run's calls, so
        # the Q7 descgen trims them (no descriptors, slot left stale).
        idx_local = np.zeros(GT * P, dtype=np.int16)
        ind_g = np.zeros(GT * P, dtype=np.int32)  # flat (pos -> dst col) helper
        valid = np.zeros(GT * P, dtype=bool)
        for t in range(NT):
            for w in range(NW):
                n = cnt[t, w]
                if n == 0 and G[t, w] == 0:
                    continue
                g0 = g_of_run[(t, w)]
                pos0 = g0 * P
                o = run_off[t, w]
                sl = slice(pos0, pos0 + n)
                idx_local[sl] = (es[o : o + n] - w * WIN).astype(np.int16)
                ind_g[sl] = ed[o : o + n] - t * P  # dst col within tile
                valid[sl] = True
        # indicator fp8 [128, GT*128]: part=edge slot%128, col=g*128+dstcol
        ind = np.zeros((P, GT * P), dtype=FP8)
        pos = np.nonzero(valid)[0]
        gidx = pos // P
        prt = pos % P
        ind[prt, gidx * P + ind_g[pos]] = FP8(1.0)
        # idx16 wrap: idx #i at [i%16, i//16], replicated to partitions 16-31
        tot = GT * P
        arr = idx_local.reshape(tot // 16, 16).T  # [16, tot/16]
        idx16 = np.concatenate([arr, arr, arr, arr], axis=0).copy()
        # dinv tiles [128, NT] (slot p of tile t = node t*128+p), pad -> 0
        dv = np.zeros((P, NT), dtype=np.float32)
        flat = np.zeros(SLOTS, dtype=np.float32)
        flat[:SHARD] = dinv[k * SHARD : (k + 1) * SHARD]
        dv[:, :] = flat.reshape(NT, P).T
        per_core.append(dict(idx16=idx16, ind=ind, dinv=dv))

    plan = dict(
        G=G, GT=GT, calls=calls, group_tile=group_tile,
        start_flag=start_flag, stop_flag=stop_flag, dinv=dinv,
        per_core=per_core,
    )
    return plan


# ---------------------------------------------------------------- launch 1
def _build_l1():
    nc = bacc.Bacc(name="gcn_l1")
    xT = nc.dram_tensor("xT", [P, SLOTS], mybir.dt.bfloat16, kind="ExternalInput")
    w1 = nc.dram_tensor("w1", [IN_C, H2], mybir.dt.bfloat16, kind="ExternalInput")
    dv = nc.dram_tensor("dv", [P, NT], mybir.dt.float32, kind="ExternalInput")
    g1 = nc.dram_tensor("g1", [P, NT, H2], mybir.dt.bfloat16, kind="ExternalOutput")

    with tile.TileContext(nc) as tc:
        with (
            tc.tile_pool(name="sbuf", bufs=1) as pool,
            tc.tile_pool(name="psum", bufs=4, space="PSUM") as psum,
        ):
            xt_s = pool.tile([P, SLOTS], mybir.dt.bfloat16)
            w1_s = pool.tile([IN_C, H2], mybir.dt.bfloat16)
            dv_s = pool.tile([P, NT], mybir.dt.float32)
            g1_s = pool.tile([P, NT, H2], mybir.dt.bfloat16)
            nc.sync.dma_start(out=xt_s[:], in_=xT[:])
            nc.sync.dma_start(out=w1_s[:], in_=w1[:])
            nc.sync.dma_start(out=dv_s[:], in_=dv[:])
            for t in range(NT):
                acc = psum.tile([P, H2], mybir.dt.float32, name=f"ps{t}", tag="ps", bufs=4)
                nc.tensor.matmul(
                    out=acc[:],
                    lhsT=xt_s[:, t * P : (t + 1) * P],
                    rhs=w1_s[:],
                    start=True,
                    stop=True,
                )
                nc.scalar.activation(
                    out=g1_s[:, t, :],
                    in_=acc[:],
                    func=mybir.ActivationFunctionType.Copy,
                    scale=dv_s[:, t : t + 1],
                )
            nc.sync.dma_start(out=g1[:], in_=g1_s[:])
    nc.compile()
    return nc


# ------------------------------------------------------- launches 2 and 3
def _build_agg(plan, layer):
    """layer=2: rhs feat 128, h1->W2->g2 chain, out g2 bf16 [P, NT, HID].
    layer=3: rhs feat 64 (slice of 128-col bf16 table), out f32 [P, NT, HID]."""
    GT = plan["GT"]
    calls = plan["calls"]
    gtile = plan["group_tile"]
    startf = plan["start_flag"]
    stopf = plan["stop_flag"]
    feat = H2 if layer == 2 else HID

    nc = bacc.Bacc(name=f"gcn_l{layer}")
    gtab = nc.dram_tensor("gtab", [N_NODES, H2], mybir.dt.bfloat16, kind="ExternalInput")
    idx = nc.dram_tensor("idx", [64, GT * P // 16], mybir.dt.int16, kind="ExternalInput")
    indt = nc.dram_tensor("indt", [P, GT * P], mybir.dt.float8e4, kind="ExternalInput")
    dv = nc.dram_tensor("dv", [P, NT], mybir.dt.float32, kind="ExternalInput")
    if layer == 2:
        w2 = nc.dram_tensor("w2", [H2, HID], mybir.dt.bfloat16, kind="ExternalInput")
        idn = nc.dram_tensor("idn", [P, P], mybir.dt.bfloat16, kind="ExternalInput")
        outT = nc.dram_tensor("outT", [P, NT, HID], mybir.dt.bfloat16, kind="ExternalOutput")
    else:
        outT = nc.dram_tensor("outT", [P, NT, HID], mybir.dt.float32, kind="ExternalOutput")

    with tile.TileContext(nc) as tc:
        with (
            tc.tile_pool(name="sbuf", bufs=1) as pool,
            tc.tile_pool(name="psum", bufs=1, space="PSUM") as psum,
        ):
            nc.gpsimd.load_library(library_config.mlp)
            idx_s = pool.tile([64, GT * P // 16], mybir.dt.int16)
            dv_s = pool.tile([P, NT], mybir.dt.float32)
            nc.sync.dma_start(out=idx_s[:], in_=idx[:])
            nc.sync.dma_start(out=dv_s[:], in_=dv[:])
            if layer == 2:
                w2_s = pool.tile([H2, HID], mybir.dt.bfloat16)
                nc.sync.dma_start(out=w2_s[:], in_=w2[:])
                idt = pool.tile([P, P], mybir.dt.bfloat16)
                nc.sync.dma_start(out=idt[:], in_=idn[:])
                out_s = pool.tile([P, NT, HID], mybir.dt.bfloat16)
            else:
                out_s = pool.tile([P, NT, HID], mybir.dt.float32)

            # psum accumulators: 4 rotating tiles (one per tile-in-batch)
            accs = {}

            def acc_for(t):
                if t not in accs:
                    accs[t] = psum.tile(
                        [P, feat], mybir.dt.float32,
                        name=f"acc{t}", tag=f"acc{t % TBATCH}", bufs=1,
                    )
                return accs[t]

            # staging buffers; memset once (pad slots are never gathered, so
            # they must hold finite values for the x0-indicator matmuls)
            NMSG = 6
            msgbufs = []
            indbufs = []
            for j in range(NMSG):
                mb = pool.tile([P, MAXG_CALL, H2], mybir.dt.bfloat16, name=f"msgb{j}")
                nc.vector.memset(mb[:], 0)
                msgbufs.append(mb)
                indbufs.append(
                    pool.tile([P, MAXG_CALL * P], mybir.dt.float8e4, name=f"indb{j}")
                )

            for ci, (w, goff, gc) in enumerate(calls):
                msg = msgbufs[ci % NMSG]
                ind_c = indbufs[ci % NMSG]
                nidx = gc * P
                nc.sync.dma_start(
                    out=ind_c[:, : gc * P],
                    in_=indt[:, goff * P : (goff + gc) * P],
                )
                nc.gpsimd.dma_gather(
                    out_ap=msg[:, :gc, :],
                    in_ap=gtab[w * WIN : (w + 1) * WIN, :],
                    idxs_ap=idx_s[:, goff * P // 16 : (goff + gc) * P // 16],
                    num_idxs=nidx,
                    num_idxs_reg=nidx,
                    elem_size=H2,
                    single_packet=True,
                )
                for gl in range(gc):
                    g = goff + gl
                    t = int(gtile[g])
                    nc.tensor.matmul(
                        out=acc_for(t)[:],
                        lhsT=ind_c[:, gl * P : (gl + 1) * P],
                        rhs=msg[:, gl, :feat],
                        start=bool(startf[g]),
                        stop=bool(stopf[g]),
                    )
                    if stopf[g]:
                        # evict chain for tile t
                        acc = accs.pop(t)
                        if layer == 2:
                            tmp = pool.tile(
                                [P, H2], mybir.dt.bfloat16,
                                name=f"h1_{t}", tag="h1", bufs=3,
                            )
                            # h1 = relu(acc * dinv)   (b1 == 0)
                            nc.scalar.activation(
                                out=tmp[:], in_=acc[:],
                                func=mybir.ActivationFunctionType.Relu,
                                scale=dv_s[:, t : t + 1],
                            )
                            tp = psum.tile(
                                [P, P], mybir.dt.bfloat16,
                                name=f"tp{t}", tag="tp", bufs=2,
                            )
                            nc.tensor.transpose(out=tp[:], in_=tmp[:], identity=idt[:])
                            h1T = pool.tile(
                                [P, P], mybir.dt.bfloat16,
                                name=f"h1T_{t}", tag="h1T", bufs=3,
                            )
                            nc.scalar.activation(
                                out=h1T[:], in_=tp[:],
                                func=mybir.ActivationFunctionType.Copy,
                            )
                            g2p = psum.tile(
                                [P, HID], mybir.dt.float32,
                                name=f"g2p{t}", tag="g2p", bufs=2,
                            )
                            nc.tensor.matmul(
                                out=g2p[:], lhsT=h1T[:], rhs=w2_s[:],
                                start=True, stop=True,
                            )
                            nc.scalar.activation(
                                out=out_s[:, t, :], in_=g2p[:],
                                func=mybir.ActivationFunctionType.Copy,
                                scale=dv_s[:, t : t + 1],
                            )
                        else:
                            # out2 = acc * dinv   (b2 == 0)
                            nc.scalar.activation(
                                out=out_s[:, t, :], in_=acc[:],
                                func=mybir.ActivationFunctionType.Copy,
                                scale=dv_s[:, t : t + 1],
                            )
            nc.sync.dma_start(out=outT[:], in_=out_s[:])
    nc.compile()
    return nc


# ---------------------------------------------------------------- kernel
def kernel(x, edge_index, W1, b1, W2, b2):
    x = np.asarray(x)
    W1 = np.asarray(W1)
    b1 = np.asarray(b1)
    W2 = np.asarray(W2)
    b2 = np.asarray(b2)
    assert not b1.any() and not b2.any(), "nonzero bias unsupported fast path"

    plan = _prepare(np.asarray(edge_index))
    dinv = plan["dinv"]

    # ---- launch 1
    nc1 = _build_l1()
    xT = np.ascontiguousarray(x.T).astype(BF16)  # [128, 100000]
    w1b = W1.astype(BF16)
    in1 = []
    for k in range(NCORES):
        sh = np.zeros((P, SLOTS), dtype=BF16)
        sh[:, :SHARD] = xT[:, k * SHARD : (k + 1) * SHARD]
        in1.append({"xT": sh, "w1": w1b, "dv": plan["per_core"][k]["dinv"]})
    r1 = run_bass_kernel_spmd(nc1, in1, core_ids=list(range(NCORES)))

    g1 = np.zeros((N_NODES, H2), dtype=BF16)
    for k in range(NCORES):
        o = r1.results[k]["g1"]  # [P, NT, H2]
        g1[k * SHARD : (k + 1) * SHARD] = (
            o.transpose(1, 0, 2).reshape(SLOTS, H2)[:SHARD]
        )

    # ---- launch 2
    nc2 = _build_agg(plan, 2)
    w2b = W2.astype(BF16)
    in2 = []
    for k in range(NCORES):
        pc = plan["per_core"][k]
        in2.append(
            {"gtab": g1, "idx": pc["idx16"], "indt": pc["ind"],
             "dv": pc["dinv"], "w2": w2b, "idn": np.eye(P, dtype=BF16)}
        )
    r2 = run_bass_kernel_spmd(nc2, in2, core_ids=list(range(NCORES)))

    g2 = np.zeros((N_NODES, H2), dtype=BF16)
    for k in range(NCORES):
        o = r2.results[k]["outT"]  # [P, NT, HID] bf16
        g2[k * SHARD : (k + 1) * SHARD, :HID] = (
            o.transpose(1, 0, 2).reshape(SLOTS, HID)[:SHARD]
        )

    # ---- launch 3
    nc3 = _build_agg(plan, 3)
    in3 = []
    for k in range(NCORES):
        pc = plan["per_core"][k]
        in3.append(
            {"gtab": g2, "idx": pc["idx16"], "indt": pc["ind"], "dv": pc["dinv"]}
        )
    r3 = run_bass_kernel_spmd(nc3, in3, core_ids=list(range(NCORES)))

    out = np.zeros((N_NODES, HID), dtype=np.float32)
    for k in range(NCORES):
        o = r3.results[k]["outT"]  # [P, NT, HID] f32
        out[k * SHARD : (k + 1) * SHARD] = (
            o.transpose(1, 0, 2).reshape(SLOTS, HID)[:SHARD]
        )
    return out



# revision 14
# speedup vs baseline: 209.2501x; 209.2501x over previous
"""2-layer GCN encoder on 8 Trainium2 NeuronCores.

Strategy (dst-tile-sharded graph parallel, 2 SPMD launches):
  By linearity, the per-layer weight matmul commutes with aggregation:
    conv(x) = dinv . A^T (dinv . x) @ W, so each launch gathers raw table
  rows (dma_gather, 4 SWDGE queues in parallel), segment-sums them via fp8
  one-hot indicator matmuls into PSUM per 128-dst tile, and applies the
  dense W at psum eviction (scale dinv -> PE transpose -> @W -> act).
    Launch A: table = dinv.x (bf16), evict chain ends in relu -> g1 rows.
    Launch B: table = g1 (bf16), evict chain ends in f32 output rows.
  Self-loop contributions are dense (identity-matmul of the core's own
  table rows) instead of gathered. dst tiles are assigned to cores by
  matching group-count profiles, which shrinks the shared SPMD max-over-
  cores group structure and balances gather volume.
Host relays the g table between launches (the all-to-all) and precomputes
the edge structure (sort, windows, indicators) in numpy from edge_index.
"""

import sys

sys.path.insert(0, "/opt/trn_rl_repo")

import ml_dtypes
import numpy as np

from concourse import bacc, bass, library_config, mybir, tile
from concourse.bass_utils import run_bass_kernel_spmd

N_NODES = 100000
IN_C = 128
H2 = 128  # 2*hid
HID = 64
NCORES = 8
P = 128
NTG = 784  # global 128-node tiles (node space padded to 100352)
NT = NTG // NCORES  # 98 schedule slots per core
NPAD = NTG * P
WIN = 25000
NW = 4
TBATCH = 4  # slots per psum batch (one accumulator per psum bank)
MAXG = 32  # groups per dma_gather call (4096 idx)
NMSG = 5  # rotating gather staging buffers
NQ = 4  # SWDGE queues (Q7 core pairs) used round-robin
SCRATCH = 32768

BF16 = ml_dtypes.bfloat16
FP8 = ml_dtypes.float8_e4m3


# ---------------------------------------------------------------- host prep
def _prepare(edge_index):
    src = np.asarray(edge_index[0], dtype=np.int64)
    dst = np.asarray(edge_index[1], dtype=np.int64)
    deg = np.bincount(dst, minlength=N_NODES).astype(np.float32) + 1.0
    dinv = np.zeros(NPAD, dtype=np.float32)
    dinv[:N_NODES] = 1.0 / np.sqrt(deg)

    tile_g = dst // P  # [E] global tile of each edge
    win = src // WIN
    key = tile_g * NW + win
    cnt_tw = np.bincount(key, minlength=NTG * NW).reshape(NTG, NW)
    ceil_tw = -(-cnt_tw // P)  # [NTG, NW]

    # slot assignment: sort tiles by ceil profile; chunks of 8 share a slot
    order = np.lexsort(
        (ceil_tw[:, 3], ceil_tw[:, 2], ceil_tw[:, 1], ceil_tw[:, 0],
         ceil_tw.sum(1))
    )
    slot_tiles = order.reshape(NT, 8)  # [NT, 8] tile ids per slot
    # within a slot, assign heaviest tile to least-loaded core
    T = np.zeros((NCORES, NT), dtype=np.int64)  # core, slot -> global tile
    load = np.zeros(NCORES, dtype=np.int64)
    for s in range(NT):
        tiles = slot_tiles[s]
        sz = cnt_tw[tiles].sum(1)
        avail = list(range(NCORES))
        for t_id in tiles[np.argsort(-sz)]:
            k = avail[int(np.argmin(load[avail]))]
            avail.remove(k)
            T[k, s] = t_id
            load[k] += cnt_tw[t_id].sum()

    G = ceil_tw[slot_tiles].max(axis=1)  # [NT, NW] shared group counts

    # batches and global group order: for b: for w: for s in b: G[s,w] groups
    batches = [range(b, min(b + TBATCH, NT)) for b in range(0, NT, TBATCH)]
    group_slot = []
    calls_by_batch = []  # per batch: list of (w, goff, gc)
    for slots_b in batches:
        bcalls = []
        for w in range(NW):
            seg = []
            for s in slots_b:
                seg += [s] * int(G[s, w])
            goff = len(group_slot)
            for cs in range(0, len(seg), MAXG):
                bcalls.append((w, goff + cs, min(MAXG, len(seg) - cs)))
            group_slot += seg
        calls_by_batch.append(bcalls)
    group_slot = np.array(group_slot, dtype=np.int64)
    GT = len(group_slot)
    calls = [c for bc in calls_by_batch for c in bc]

    # group start offset of each (s, w) run
    g_of_run = np.full((NT, NW), -1, dtype=np.int64)
    gi = 0
    for slots_b in batches:
        for w in range(NW):
            for s in slots_b:
                g_of_run[s, w] = gi
                gi += G[s, w]
    assert gi == GT

    # stop flags: last group of each slot (identity matmul opens the acc)
    stop_flag = np.zeros(GT, dtype=bool)
    id_stop = np.zeros(NT, dtype=bool)
    for s in range(NT):
        gs = np.nonzero(group_slot == s)[0]
        if len(gs):
            stop_flag[gs[-1]] = True
        else:
            id_stop[s] = True

    # per-core idx16 / indicators / dinv / own-row order
    per_core = []
    for k in range(NCORES):
        tile_of_edge = np.full(NTG, -1, dtype=np.int64)
        tile_of_edge[T[k]] = np.arange(NT)
        m = tile_of_edge[tile_g] >= 0
        es, ed, ew = src[m], dst[m], win[m]
        slot = tile_of_edge[tile_g[m]]
        o = np.lexsort((ed, ew, slot))
        es, ed, ew, slot = es[o], ed[o], ew[o], slot[o]
        cnt = cnt_tw[T[k]]  # [NT, NW]
        run_off = np.concatenate([[0], np.cumsum(cnt.reshape(-1))])[:-1].reshape(
            NT, NW
        )

        idx_local = np.zeros(GT * P, dtype=np.int16)  # pads stay 0 (safe row)
        ind = np.zeros((P, GT * P), dtype=FP8)
        for s in range(NT):
            for w in range(NW):
                n = cnt[s, w]
                if n == 0:
                    continue
                pos0 = g_of_run[s, w] * P
                o0 = run_off[s, w]
                sl = slice(pos0, pos0 + n)
                idx_local[sl] = (es[o0 : o0 + n] - w * WIN).astype(np.int16)
                cols = ed[o0 : o0 + n] - T[k, s] * P
                pos = np.arange(pos0, pos0 + n)
                ind[pos % P, (pos // P) * P + cols] = FP8(1.0)
        arr = idx_local.reshape(GT * P // 16, 16).T  # [16, GT*8]
        idx16 = np.concatenate([arr] * 8, axis=0).copy()  # [128, GT*8]
        dv = dinv[(T[k][None, :] * P + np.arange(P)[:, None])].astype(
            np.float32
        )  # [128, NT]
        per_core.append(dict(idx16=idx16, ind=ind, dinv=dv))

    return dict(
        G=G, GT=GT, calls=calls, calls_by_batch=calls_by_batch,
        group_slot=group_slot, stop_flag=stop_flag,
        id_stop=id_stop, dinv=dinv, T=T, per_core=per_core,
    )


# --------------------------------------------------------------- arrange
def _arrange_own(table_pad, T):
    """[NPAD, 128] -> per-core [128, NT, 128] own-tile rows (node = s*128+p)."""
    outs = []
    for k in range(NCORES):
        rows = table_pad[(T[k][:, None] * P + np.arange(P)[None, :]).reshape(-1)]
        outs.append(
            np.ascontiguousarray(rows.reshape(NT, P, -1).transpose(1, 0, 2))
        )
    return outs


# ---------------------------------------------------------------- launches
def _build(plan, layer):
    """layer=1: table=xs, evict relu(@W1) -> bf16 g1 rows.
    layer=2: table=g1, evict @W2 -> f32 out rows."""
    GT = plan["GT"]
    calls = plan["calls"]
    gslot = plan["group_slot"]
    stopf = plan["stop_flag"]
    id_stop = plan["id_stop"]
    feat_out = H2 if layer == 1 else HID

    nc = bacc.Bacc(
        name=f"gcn_l{layer}",
        dynamic_dma_scratch_size=SCRATCH,
        num_swdge_queues=NQ,
    )
    gtab = nc.dram_tensor(
        "gtab", [N_NODES, H2], mybir.dt.bfloat16, kind="ExternalInput"
    )
    idx = nc.dram_tensor("idx", [128, GT * 8], mybir.dt.int16, kind="ExternalInput")
    indt = nc.dram_tensor("indt", [P, GT * P], mybir.dt.float8e4, kind="ExternalInput")
    own = nc.dram_tensor("own", [P, NT, H2], mybir.dt.bfloat16, kind="ExternalInput")
    dv = nc.dram_tensor("dv", [P, NT], mybir.dt.float32, kind="ExternalInput")
    wmat = nc.dram_tensor(
        "wmat", [H2, feat_out], mybir.dt.bfloat16, kind="ExternalInput"
    )
    idn = nc.dram_tensor("idn", [P, P], mybir.dt.bfloat16, kind="ExternalInput")
    idf = nc.dram_tensor("idf", [P, P], mybir.dt.float8e4, kind="ExternalInput")
    odt = mybir.dt.bfloat16 if layer == 1 else mybir.dt.float32
    outT = nc.dram_tensor("outT", [P, NT, feat_out], odt, kind="ExternalOutput")

    with tile.TileContext(nc) as tc:
        with (
            tc.tile_pool(name="sbuf", bufs=1) as pool,
            tc.tile_pool(name="psum", bufs=1, space="PSUM") as psum,
        ):
            nc.gpsimd.load_library(library_config.mlp)
            idx_s = pool.tile([128, GT * 8], mybir.dt.int16)
            dv_s = pool.tile([P, NT], mybir.dt.float32)
            own_s = pool.tile([P, NT, H2], mybir.dt.bfloat16)
            w_s = pool.tile([H2, feat_out], mybir.dt.bfloat16)
            idn_s = pool.tile([P, P], mybir.dt.bfloat16)
            idf_s = pool.tile([P, P], mybir.dt.float8e4)
            nc.sync.dma_start(out=idx_s[:], in_=idx[:])
            nc.sync.dma_start(out=dv_s[:], in_=dv[:])
            nc.sync.dma_start(out=own_s[:], in_=own[:])
            nc.sync.dma_start(out=w_s[:], in_=wmat[:])
            nc.sync.dma_start(out=idn_s[:], in_=idn[:])
            nc.sync.dma_start(out=idf_s[:], in_=idf[:])
            out_s = pool.tile([P, NT, feat_out], odt)

            msgbufs = [
                pool.tile([P, MAXG, H2], mybir.dt.bfloat16, name=f"msg{j}")
                for j in range(NMSG)
            ]
            indbufs = [
                pool.tile([P, MAXG * P], mybir.dt.float8e4, name=f"ind{j}")
                for j in range(NMSG)
            ]

            accs = {}

            def acc_for(s):
                if s not in accs:
                    accs[s] = psum.tile(
                        [P, H2], mybir.dt.float32,
                        name=f"acc{s}", tag=f"acc{s % TBATCH}", bufs=1,
                    )
                return accs[s]

            def evict(s):
                acc = accs.pop(s)
                t0 = pool.tile(
                    [P, H2], mybir.dt.bfloat16, name=f"t0_{s}", tag="t0", bufs=3
                )
                nc.scalar.activation(
                    out=t0[:], in_=acc[:],
                    func=mybir.ActivationFunctionType.Copy,
                    scale=dv_s[:, s : s + 1],
                )
                tp = psum.tile(
                    [P, P], mybir.dt.bfloat16, name=f"tp{s}", tag="tp", bufs=2
                )
                nc.tensor.transpose(out=tp[:], in_=t0[:], identity=idn_s[:])
                t0T = pool.tile(
                    [P, P], mybir.dt.bfloat16, name=f"t0T_{s}", tag="t0T", bufs=3
                )
                nc.scalar.copy(out=t0T[:], in_=tp[:])
                hp = psum.tile(
                    [P, feat_out], mybir.dt.float32,
                    name=f"hp{s}", tag="mo", bufs=2,
                )
                nc.tensor.matmul(
                    out=hp[:], lhsT=t0T[:], rhs=w_s[:], start=True, stop=True
                )
                if layer == 1:
                    nc.scalar.activation(
                        out=out_s[:, s, :], in_=hp[:],
                        func=mybir.ActivationFunctionType.Relu,
                        scale=dv_s[:, s : s + 1],
                    )
                else:
                    nc.scalar.copy(out=out_s[:, s, :], in_=hp[:])

            # schedule: per batch, identities first, then calls/groups
            cj = 0
            for bi, slots_b in enumerate(
                range(b0, min(b0 + TBATCH, NT))
                for b0 in range(0, NT, TBATCH)
            ):
                for s in slots_b:
                    nc.tensor.matmul(
                        out=acc_for(s)[:],
                        lhsT=idf_s[:],
                        rhs=own_s[:, s, :],
                        start=True,
                        stop=bool(id_stop[s]),
                        skip_group_check=True,
                    )
                    if id_stop[s]:
                        evict(s)
                for w, goff, gc in plan["calls_by_batch"][bi]:
                    msg = msgbufs[cj % NMSG]
                    ind_c = indbufs[cj % NMSG]
                    nc.sync.dma_start(
                        out=ind_c[:, : gc * P],
                        in_=indt[:, goff * P : (goff + gc) * P],
                    )
                    nidx = gc * P
                    nc.gpsimd.dma_gather(
                        out_ap=msg[:, :gc, :],
                        in_ap=gtab[w * WIN : (w + 1) * WIN, :],
                        idxs_ap=idx_s[:, goff * 8 : (goff + gc) * 8],
                        num_idxs=nidx,
                        num_idxs_reg=nidx,
                        elem_size=H2,
                        single_packet=(nidx <= 1024),
                        queue_num=cj % NQ,
                    )
                    for gl in range(gc):
                        g = goff + gl
                        s = int(gslot[g])
                        nc.tensor.matmul(
                            out=acc_for(s)[:],
                            lhsT=ind_c[:, gl * P : (gl + 1) * P],
                            rhs=msg[:, gl, :],
                            start=False,
                            stop=bool(stopf[g]),
                            skip_group_check=True,
                        )
                        if stopf[g]:
                            evict(s)
                    cj += 1
            assert cj == len(calls) and not accs, (cj, len(calls), accs.keys())
            nc.sync.dma_start(out=outT[:], in_=out_s[:])
    nc.compile()
    return nc


# ---------------------------------------------------------------- kernel
def kernel(x, edge_index, W1, b1, W2, b2):
    x = np.asarray(x)
    W1 = np.asarray(W1)
    b1 = np.asarray(b1)
    W2 = np.asarray(W2)
    b2 = np.asarray(b2)
    assert not b1.any() and not b2.any(), "nonzero bias unsupported fast path"

    plan = _prepare(np.asarray(edge_index))
    dinv = plan["dinv"]  # [NPAD]
    T = plan["T"]

    xs_pad = np.zeros((NPAD, IN_C), dtype=np.float32)
    xs_pad[:N_NODES] = x * dinv[:N_NODES, None]
    xs_pad = xs_pad.astype(BF16)
    own1 = _arrange_own(xs_pad, T)

    idn = np.eye(P, dtype=BF16)
    idf = np.eye(P, dtype=FP8)
    w1b = W1.astype(BF16)

    nc1 = _build(plan, 1)
    in1 = []
    for k in range(NCORES):
        pc = plan["per_core"][k]
        in1.append(
            {"gtab": xs_pad[:N_NODES], "idx": pc["idx16"], "indt": pc["ind"],
             "own": own1[k], "dv": pc["dinv"], "wmat": w1b, "idn": idn,
             "idf": idf}
        )
    r1 = run_bass_kernel_spmd(nc1, in1, core_ids=list(range(NCORES)))

    g1_pad = np.zeros((NPAD, H2), dtype=BF16)
    for k in range(NCORES):
        o = r1.results[k]["outT"]  # [P, NT, H2]
        g1_pad[(T[k][:, None] * P + np.arange(P)[None, :]).reshape(-1)] = (
            o.transpose(1, 0, 2).reshape(NT * P, H2)
        )
    own2 = _arrange_own(g1_pad, T)

    nc2 = _build(plan, 2)
    w2b = W2.astype(BF16)
    in2 = []
    for k in range(NCORES):
        pc = plan["per_core"][k]
        in2.append(
            {"gtab": g1_pad[:N_NODES], "idx": pc["idx16"], "indt": pc["ind"],
             "own": own2[k], "dv": pc["dinv"], "wmat": w2b, "idn": idn,
             "idf": idf}
        )
    r2 = run_bass_kernel_spmd(nc2, in2, core_ids=list(range(NCORES)))

    out = np.zeros((N_NODES, HID), dtype=np.float32)
    out_pad = np.zeros((NPAD, HID), dtype=np.float32)
    for k in range(NCORES):
        o = r2.results[k]["outT"]  # [P, NT, HID] f32
        out_pad[(T[k][:, None] * P + np.arange(P)[None, :]).reshape(-1)] = (
            o.transpose(1, 0, 2).reshape(NT * P, HID)
        )
    out[:] = out_pad[:N_NODES]
    return out
